# revision 1
# baseline (speedup 1.0000x reference)
"""BertEmbeddings (word lookup + header mean-pool scatter + pos/type/match
embeddings + TF-style LayerNorm) as a Bass/Tile kernel on 8 trn2 NeuronCores.

Sharding: data-parallel over batch (4 rows/core); embedding tables replicated.

Per-core device pipeline:
  - dma_gather word rows for the core's 2048 tokens      -> [128, 16, 768]
  - dma_gather header rows in (l, slot) order, with pad slots pointed at an
    all-zero row appended to the word table               -> 4x [128, 4, 768]
  - pool headers: sum over l (free-dim adds) * 1/max(len,1) -> pooled [128 slots, 768]
  - small tables (tok_type/match/type concat [19,768]) added via a multi-hot
    matmul; the pooled-row scatter is done in the same PSUM accumulation with
    a one-hot (slot -> target token) matmul whose extra ones-column also
    yields the scatter mask.
  - blend, add positional rows, LayerNorm (bn_stats/bn_aggr), store.

All data-dependent arithmetic on embedding VALUES runs on device; the host
only reformats index tensors (gather index layout, zero-row padding,
slot permutation by col_idx, scatter target columns).
"""

import numpy as np

B, S, H = 32, 512, 768
VOCAB = 30522
NCORES = 8
BPC = B // NCORES            # batch rows per core
T = BPC * S                  # tokens per core
NBLK = T // 128              # 128-token blocks per core
C, L = 32, 16                # columns, max header len
NSLOT = BPC * C              # 128 slots per core
ZROW = VOCAB                 # zero row in augmented word table
WROWS = VOCAB + 1
NV = 19                      # 2 + 11 + 6 small-table rows
EPS = 1e-12

_NC_CACHE = {}


def _build_nc(skip_affine: bool):
    from contextlib import ExitStack

    import concourse.bacc as bacc
    import concourse.tile as tile
    from concourse import mybir

    F32 = mybir.dt.float32
    I16 = mybir.dt.int16

    nc = bacc.Bacc("TRN2", target_bir_lowering=False, debug=False)
    t = {}

    def inp(name, shape, dt=F32):
        t[name] = nc.dram_tensor(name, shape, dt, kind="ExternalInput").ap()

    inp("word_aug", [WROWS, H])
    inp("pos_emb", [S, H])
    inp("small3", [NV, H])
    if not skip_affine:
        inp("lnw", [1, H])
        inp("lnb", [1, H])
    inp("iota_t", [1, T])
    inp("iota19", [NV, 1])
    inp("ids3f", [3, T])
    inp("tgt", [128, 1])
    inp("hl", [128, 1])
    inp("widx", [128, T // 16], I16)
    inp("hidx", [128, T // 16], I16)
    out = nc.dram_tensor("out", [BPC, S, H], F32, kind="ExternalOutput").ap()

    with tile.TileContext(nc) as tc, ExitStack() as ctx:
        _body(ctx, tc, t, out, skip_affine, mybir)
    nc.compile()
    return nc


def _body(ctx, tc, t, out, skip_affine, mybir):
    nc = tc.nc
    F32 = mybir.dt.float32
    I16 = mybir.dt.int16
    EQ = mybir.AluOpType.is_equal
    MUL = mybir.AluOpType.mult
    ADD = mybir.AluOpType.add
    SUB = mybir.AluOpType.subtract

    const = ctx.enter_context(tc.tile_pool(name="const", bufs=1))
    setup = ctx.enter_context(tc.tile_pool(name="setup", bufs=1))
    hpool = ctx.enter_context(tc.tile_pool(name="hdr", bufs=3))
    h2pool = ctx.enter_context(tc.tile_pool(name="h2", bufs=2))
    h1pool = ctx.enter_context(tc.tile_pool(name="h1", bufs=2))
    wpool = ctx.enter_context(tc.tile_pool(name="wrd", bufs=3))
    epool = ctx.enter_context(tc.tile_pool(name="emb", bufs=3))
    spool = ctx.enter_context(tc.tile_pool(name="stat", bufs=4))
    psum = ctx.enter_context(tc.tile_pool(name="ps", bufs=3, space="PSUM"))

    # ---------------- constants / index tiles ----------------
    s_widx = const.tile([128, T // 16], I16)
    nc.sync.dma_start(s_widx[:], t["widx"])
    s_hidx = const.tile([128, T // 16], I16)
    nc.sync.dma_start(s_hidx[:], t["hidx"])

    s_small = const.tile([NV, H + 1], F32)
    nc.sync.dma_start(s_small[:, 0:H], t["small3"])
    nc.vector.memset(s_small[:, H : H + 1], 0.0)

    s_pos = const.tile([128, BPC, H], F32)
    nc.sync.dma_start(s_pos[:], t["pos_emb"].rearrange("(j p) h -> p j h", p=128))

    s_eps = const.tile([128, 1], F32)
    nc.vector.memset(s_eps[:], EPS)

    if not skip_affine:
        s_lnw = const.tile([128, H], F32)
        nc.gpsimd.dma_start(s_lnw[:], t["lnw"].partition_broadcast(128).opt([1]))
        s_lnb = const.tile([128, H], F32)
        nc.gpsimd.dma_start(s_lnb[:], t["lnb"].partition_broadcast(128).opt([1]))

    s_tgt = const.tile([128, 1], F32)
    nc.sync.dma_start(s_tgt[:], t["tgt"])
    s_hl = const.tile([128, 1], F32)
    nc.sync.dma_start(s_hl[:], t["hl"])
    s_recip = const.tile([128, 1], F32)
    nc.vector.tensor_scalar_max(s_recip[:], s_hl[:], 1.0)
    nc.vector.reciprocal(s_recip[:], s_recip[:])

    s_i19 = const.tile([NV, 1], F32)
    nc.sync.dma_start(s_i19[:], t["iota19"])

    # multi-hot [19, T] for the three small tables
    s_idsb = setup.tile([NV, 3, T], F32)
    nc.gpsimd.dma_start(s_idsb[:], t["ids3f"].partition_broadcast(NV))
    nc.vector.tensor_scalar(s_idsb[:], s_idsb[:], s_i19[:], None, op0=EQ)
    s_mh = const.tile([NV, T], F32)
    nc.vector.tensor_add(s_mh[:], s_idsb[:, 0, :], s_idsb[:, 1, :])
    nc.vector.tensor_add(s_mh[:], s_mh[:], s_idsb[:, 2, :])

    # one-hot [128 slots, T] scatter matrix (column tgt[k], invalid -> none)
    s_iotat = const.tile([128, T], F32)
    nc.gpsimd.dma_start(s_iotat[:], t["iota_t"].partition_broadcast(128).opt([1]))
    s_oh = const.tile([128, T], F32)
    nc.vector.tensor_scalar(s_oh[:], s_iotat[:], s_tgt[:], None, op0=EQ)

    # ---------------- header gather + pooling ----------------
    # gather order i2 = l*128 + slot  ->  hch[:, m, :] holds l = 4*lc + m
    hsum = const.tile([128, H + 1], F32)
    hacc = setup.tile([128, H], F32)
    for lc in range(4):
        hch = hpool.tile([128, 4, H], F32)
        nc.gpsimd.dma_gather(
            hch[:], t["word_aug"], s_hidx[:, 32 * lc : 32 * (lc + 1)], 512, 512, H
        )
        h2 = h2pool.tile([128, 2, H], F32)
        nc.vector.tensor_add(h2[:], hch[:, 0:2, :], hch[:, 2:4, :])
        if lc == 0:
            nc.vector.tensor_add(hacc[:], h2[:, 0, :], h2[:, 1, :])
        else:
            h1 = h1pool.tile([128, H], F32)
            nc.vector.tensor_add(h1[:], h2[:, 0, :], h2[:, 1, :])
            nc.vector.tensor_add(hacc[:], hacc[:], h1[:])
    nc.vector.tensor_scalar_mul(hsum[:, 0:H], hacc[:], s_recip[:])
    nc.vector.memset(hsum[:, H : H + 1], 1.0)

    # ---------------- token blocks ----------------
    for ch in range(BPC):
        wch = wpool.tile([128, 4, H], F32)
        nc.gpsimd.dma_gather(
            wch[:], t["word_aug"], s_widx[:, 32 * ch : 32 * (ch + 1)], 512, 512, H
        )
        for jj in range(4):
            j = ch * 4 + jj
            words_j = wch[:, jj, :]
            ps = psum.tile([128, H + 1], F32)
            lhs_mh = s_mh[:, j * 128 : (j + 1) * 128]
            lhs_oh = s_oh[:, j * 128 : (j + 1) * 128]
            nc.tensor.matmul(
                ps[:, 0:512], lhs_mh, s_small[:, 0:512], start=True, stop=False
            )
            nc.tensor.matmul(
                ps[:, 0:512], lhs_oh, hsum[:, 0:512], start=False, stop=True
            )
            nc.tensor.matmul(
                ps[:, 512 : H + 1], lhs_mh, s_small[:, 512 : H + 1],
                start=True, stop=False,
            )
            nc.tensor.matmul(
                ps[:, 512 : H + 1], lhs_oh, hsum[:, 512 : H + 1],
                start=False, stop=True,
            )

            notm = spool.tile([128, 1], F32)
            nc.vector.tensor_scalar(
                notm[:], ps[:, H : H + 1], -1.0, 1.0, op0=MUL, op1=ADD
            )
            emb = epool.tile([128, H], F32)
            nc.vector.tensor_scalar(emb[:], words_j, notm[:], None, op0=MUL)
            nc.vector.tensor_add(emb[:], emb[:], ps[:, 0:H])
            nc.vector.tensor_add(emb[:], emb[:], s_pos[:, jj, :])

            stats = spool.tile([128, 3, 6], F32)
            for g in range(3):
                nc.vector.bn_stats(stats[:, g, :], emb[:, g * 256 : (g + 1) * 256])
            mv = spool.tile([128, 2], F32)
            nc.vector.bn_aggr(mv[:], stats[:])
            std = spool.tile([128, 1], F32)
            nc.scalar.activation(
                std[:], mv[:, 1:2], mybir.ActivationFunctionType.Sqrt,
                bias=s_eps[:], scale=1.0,
            )
            nc.vector.reciprocal(std[:], std[:])
            nc.vector.tensor_scalar(
                emb[:], emb[:], mv[:, 0:1], std[:], op0=SUB, op1=MUL
            )
            if not skip_affine:
                nc.vector.tensor_mul(emb[:], emb[:], s_lnw[:])
                nc.vector.tensor_add(emb[:], emb[:], s_lnb[:])

            nc.sync.dma_start(out[ch, jj * 128 : (jj + 1) * 128, :], emb[:])


def _wrap16(flat):
    w = flat.reshape(T // 16, 16).T.astype(np.int16)
    return np.tile(w, (8, 1))


def _prep_core(core, iid, hdr, tt, mt, ti, cpos, cidx, hlen):
    b0 = core * BPC
    sl = slice(b0, b0 + BPC)
    widx = _wrap16(iid[sl].reshape(-1))

    bb = np.arange(BPC)[:, None]
    sel_hdr = hdr[sl][bb, cidx[sl]]                      # [BPC, C, L]
    sel_len = hlen[sl][bb, cidx[sl]]                     # [BPC, C]
    maskl = np.arange(L)[None, None, :] < sel_len[:, :, None]
    hvals = np.where(maskl, sel_hdr, ZROW)               # [BPC, C, L]
    hflat = hvals.reshape(NSLOT, L).T.reshape(-1)        # i2 = l*128 + slot
    hidx = _wrap16(hflat)

    tgt = np.where(
        sel_len.reshape(-1) > 0, (bb * S + cpos[sl]).reshape(-1), -1
    ).astype(np.float32).reshape(NSLOT, 1)
    hl = sel_len.reshape(NSLOT, 1).astype(np.float32)
    ids3f = np.stack(
        [tt[sl].reshape(-1), 2 + mt[sl].reshape(-1), 13 + ti[sl].reshape(-1)]
    ).astype(np.float32)
    return widx, hidx, tgt, hl, ids3f


def make_in_maps(inputs):
    inp = {k: np.asarray(v) for k, v in inputs.items()}
    word = np.ascontiguousarray(inp["word_emb"], dtype=np.float32)
    word_aug = np.concatenate([word, np.zeros((1, H), np.float32)], axis=0)
    small3 = np.ascontiguousarray(
        np.concatenate(
            [inp["tok_type_emb"], inp["match_emb"], inp["type_emb"]], axis=0
        ),
        dtype=np.float32,
    )
    pos = np.ascontiguousarray(inp["pos_emb"], dtype=np.float32)
    lnw = np.ascontiguousarray(inp["ln_w"], dtype=np.float32).reshape(1, H)
    lnb = np.ascontiguousarray(inp["ln_b"], dtype=np.float32).reshape(1, H)
    skip_affine = bool(np.all(lnw == 1.0) and np.all(lnb == 0.0))

    iota_t = np.arange(T, dtype=np.float32).reshape(1, T)
    iota19 = np.arange(NV, dtype=np.float32).reshape(NV, 1)

    iid = inp["input_ids"].astype(np.int64)
    hdr = inp["header_ids"].astype(np.int64)
    tt = inp["token_type_ids"].astype(np.int64)
    mt = inp["match_type_ids"].astype(np.int64)
    ti = inp["type_idx"].astype(np.int64)
    cpos = inp["col_pos"].astype(np.int64)
    cidx = inp["col_idx"].astype(np.int64)
    hlen = inp["header_len"].astype(np.int64)

    in_maps = []
    for core in range(NCORES):
        widx, hidx, tgt, hl, ids3f = _prep_core(
            core, iid, hdr, tt, mt, ti, cpos, cidx, hlen
        )
        m = dict(
            word_aug=word_aug, pos_emb=pos, small3=small3,
            iota_t=iota_t, iota19=iota19, ids3f=ids3f,
            tgt=tgt, hl=hl, widx=widx, hidx=hidx,
        )
        if not skip_affine:
            m["lnw"] = lnw
            m["lnb"] = lnb
        in_maps.append(m)
    return in_maps, skip_affine


def get_nc(skip_affine):
    if skip_affine not in _NC_CACHE:
        _NC_CACHE[skip_affine] = _build_nc(skip_affine)
    return _NC_CACHE[skip_affine]


def run_hw(inputs, trace=False, trace_cores=None):
    """Returns (out [B,S,H] f32, BassKernelResults)."""
    from concourse.bass_utils import run_bass_kernel_spmd

    in_maps, skip_affine = make_in_maps(inputs)
    nc = get_nc(skip_affine)
    res = run_bass_kernel_spmd(
        nc, in_maps, core_ids=list(range(NCORES)), trace=trace,
        trace_cores=trace_cores,
    )
    out = np.concatenate([res.results[c]["out"] for c in range(NCORES)], axis=0)
    return out, res


def kernel(**inputs) -> np.ndarray:
    out, _ = run_hw(inputs, trace=False)
    return out


# revision 20
# speedup vs baseline: 1.5182x; 1.5182x over previous
"""BertEmbeddings (word lookup + header mean-pool scatter + pos/type/match
embeddings + TF-style LayerNorm) as a Bass/Tile kernel on 8 trn2 NeuronCores.

Sharding: data-parallel over batch (4 rows/core); embedding tables replicated.

Per-core device pipeline (v2 — engine-balanced):
  - word rows arrive via an ACCUMULATING indirect DMA gather on top of a
    pos_emb prefill (SBUF->SBUF DMA), so words+pos costs zero engine time
  - header rows (slot order, pad slots -> zero row) gathered with dma_gather,
    mean-pooled with a free-dim add tree; pooled slot rows are converted to
    REPLACEMENT deltas (pooled - word_at_target) so the scatter needs no mask
  - small tables (tok/match/type concat [19,768]) + scatter deltas are added
    in one PSUM accumulation via bf16 one-hot matmuls (host-built one-hots)
  - emb = (words+pos) + PSUM in one tensor_tensor_reduce that also emits the
    row sum; sum of squares comes from an ACT Square pass (accum_out); the
    LN apply (x-u)*rstd runs on ACT as Identity(scale=rstd, bias=-u*rstd)

All data-dependent arithmetic on embedding VALUES runs on device; the host
only reformats index tensors (gather index layouts, zero-row padding, slot
permutation by col_idx, one-hot/multi-hot index encodings).
"""

import numpy as np

B, S, H = 32, 512, 768
VOCAB = 30522
NCORES = 8
BPC = B // NCORES            # batch rows per core
T = BPC * S                  # tokens per core
NBLK = T // 128              # 128-token blocks per core
C, L = 32, 16                # columns, max header len
NSLOT = BPC * C              # 128 slots per core
ZROW = VOCAB                 # zero row in augmented word table
WROWS = VOCAB + 1
NV = 19                      # 2 + 11 + 6 small-table rows
EPS = 1e-12

_NC_CACHE = {}

import os as _os

# bisect/tuning knobs (read at build time)
OPT_SCRATCH = int(_os.environ.get("K_SCRATCH", "16384"))
OPT_ACC_GATHER = _os.environ.get("K_ACC", "1") == "1"   # accumulate words onto pos prefill
OPT_INDIRECT = _os.environ.get("K_IND", "1") == "1"     # indirect dma for word gathers
OPT_ACT_STATS = _os.environ.get("K_ACTSTATS", "1") == "1"  # sumsq via ACT accum
OPT_ACT_APPLY = _os.environ.get("K_ACTAPPLY", "1") == "1"  # LN apply on ACT
# tensor_tensor_reduce crashes the exec unit on HW (2026-08) — keep off
OPT_TTR = _os.environ.get("K_TTR", "0") == "1"


def _build_nc(skip_affine: bool):
    from contextlib import ExitStack

    import concourse.bacc as bacc
    import concourse.tile as tile
    from concourse import mybir

    BF16 = mybir.dt.bfloat16
    I16 = mybir.dt.int16
    I32 = mybir.dt.int32
    F32 = mybir.dt.float32

    nc = bacc.Bacc(
        "TRN2", target_bir_lowering=False, debug=False,
        dynamic_dma_scratch_size=OPT_SCRATCH,
    )
    t = {}

    def inp(name, shape, dt=F32):
        t[name] = nc.dram_tensor(name, shape, dt, kind="ExternalInput").ap()

    inp("word_aug", [WROWS, H])
    inp("pos_emb", [S, H])
    inp("small3", [2 * NV, H], BF16)   # [hi; lo] error-compensated split
    if not skip_affine:
        inp("lnw", [1, H])
        inp("lnb", [1, H])
    inp("mh", [2 * NV, T], BF16)   # multi-hot small-table encoding, stacked x2
    inp("oh", [128, T], BF16)      # one-hot slot->target-column scatter matrix
    inp("hl", [128, 1])            # header_len per slot (selected by col_idx)
    inp("widx", [128, NBLK], I32)  # word row per (p, j) token
    inp("wtidx", [128, 1], I32)    # word row at each slot's target token
    inp("hidx", [128, T // 16], I16)  # header gather rows, wrap16 layout
    inp("widx16", [128, T // 16], I16)   # word rows, wrap16 (dma_gather path)
    inp("wtidx16", [128, NSLOT // 16], I16)  # target word rows, wrap16
    out = nc.dram_tensor("out", [BPC, S, H], F32, kind="ExternalOutput").ap()

    with tile.TileContext(nc) as tc, ExitStack() as ctx:
        _body(ctx, tc, t, out, skip_affine, mybir)
    nc.compile()
    return nc


def _body(ctx, tc, t, out, skip_affine, mybir):
    import concourse.bass as bass

    nc = tc.nc
    F32 = mybir.dt.float32
    BF16 = mybir.dt.bfloat16
    I16 = mybir.dt.int16
    I32 = mybir.dt.int32
    MUL = mybir.AluOpType.mult
    ADD = mybir.AluOpType.add
    AF = mybir.ActivationFunctionType

    const = ctx.enter_context(tc.tile_pool(name="const", bufs=1))
    setup = ctx.enter_context(tc.tile_pool(name="setup", bufs=1))
    hpool = ctx.enter_context(tc.tile_pool(name="hdr", bufs=3))
    h2pool = ctx.enter_context(tc.tile_pool(name="h2", bufs=2))
    h1pool = ctx.enter_context(tc.tile_pool(name="h1", bufs=2))
    wpool = ctx.enter_context(tc.tile_pool(name="wrd", bufs=3))
    epool = ctx.enter_context(tc.tile_pool(name="emb", bufs=3))
    opool = ctx.enter_context(tc.tile_pool(name="outp", bufs=3))
    qpool = ctx.enter_context(tc.tile_pool(name="sq", bufs=2))
    spool = ctx.enter_context(tc.tile_pool(name="stat", bufs=8))
    psum = ctx.enter_context(tc.tile_pool(name="ps", bufs=3, space="PSUM"))

    # ---------------- constants / index tiles ----------------
    s_widx = const.tile([128, NBLK], I32)
    nc.sync.dma_start(s_widx[:], t["widx"])
    s_wtidx = const.tile([128, 1], I32)
    nc.sync.dma_start(s_wtidx[:], t["wtidx"])
    s_hidx = const.tile([128, T // 16], I16)
    nc.sync.dma_start(s_hidx[:], t["hidx"])

    s_small = const.tile([2 * NV, H], BF16)
    nc.sync.dma_start(s_small[:], t["small3"])
    s_mh = const.tile([2 * NV, T], BF16)
    nc.sync.dma_start(s_mh[:], t["mh"])
    s_oh = const.tile([128, T], BF16)
    nc.sync.dma_start(s_oh[:], t["oh"])

    s_pos = const.tile([128, BPC, H], F32)
    nc.sync.dma_start(s_pos[:], t["pos_emb"].rearrange("(j p) h -> p j h", p=128))

    s_eps = const.tile([128, 1], F32)
    nc.vector.memset(s_eps[:], EPS)

    if not skip_affine:
        s_lnw = const.tile([128, H], F32)
        nc.gpsimd.dma_start(s_lnw[:], t["lnw"].partition_broadcast(128))
        s_lnb = const.tile([128, H], F32)
        nc.gpsimd.dma_start(s_lnb[:], t["lnb"].partition_broadcast(128))

    s_hl = const.tile([128, 1], F32)
    nc.sync.dma_start(s_hl[:], t["hl"])
    s_recip = const.tile([128, 1], F32)
    nc.vector.tensor_scalar_max(s_recip[:], s_hl[:], 1.0)
    nc.vector.reciprocal(s_recip[:], s_recip[:])

    # ---------------- header gather + pooling ----------------
    # gather order i2 = l*128 + slot  ->  hch[:, m, :] holds l = 4*lc + m
    hacc = setup.tile([128, H], F32)
    for lc in range(4):
        hch = hpool.tile([128, 4, H], F32)
        nc.gpsimd.dma_gather(
            hch[:], t["word_aug"], s_hidx[:, 32 * lc : 32 * (lc + 1)], 512, 512, H
        )
        h2 = h2pool.tile([128, 2, H], F32)
        nc.vector.tensor_add(h2[:], hch[:, 0:2, :], hch[:, 2:4, :])
        if lc == 0:
            nc.vector.tensor_add(hacc[:], h2[:, 0, :], h2[:, 1, :])
        else:
            h1 = h1pool.tile([128, H], F32)
            nc.vector.tensor_add(h1[:], h2[:, 0, :], h2[:, 1, :])
            nc.vector.tensor_add(hacc[:], hacc[:], h1[:])

    # word rows at the scatter target tokens (for replacement deltas)
    wtgt = setup.tile([128, H], F32)
    if OPT_INDIRECT:
        nc.gpsimd.indirect_dma_start(
            wtgt[:], None, t["word_aug"],
            bass.IndirectOffsetOnAxis(ap=s_wtidx[:, 0:1], axis=0),
        )
    else:
        s_wtidx16 = const.tile([128, NSLOT // 16], I16)
        nc.sync.dma_start(s_wtidx16[:], t["wtidx16"])
        wtgt3 = wtgt[:].rearrange("p (o h) -> p o h", o=1)
        nc.gpsimd.dma_gather(
            wtgt3, t["word_aug"], s_wtidx16[:], NSLOT, NSLOT, H
        )
    # hadj = pooled - word_at_target (bf16 rhs for the scatter matmul);
    # invalid slots contribute nothing (their one-hot column is all zero)
    d = setup.tile([128, H], F32)
    nc.vector.tensor_scalar_mul(d[:], hacc[:], s_recip[:])
    nc.vector.tensor_sub(d[:], d[:], wtgt[:])
    hadj_hi = setup.tile([128, H], BF16)
    nc.vector.tensor_copy(hadj_hi[:], d[:])
    hadj_lo = setup.tile([128, H], BF16)
    nc.vector.tensor_sub(hadj_lo[:], d[:], hadj_hi[:])

    s_widx16 = None
    if not OPT_INDIRECT:
        s_widx16 = const.tile([128, T // 16], I16)
        nc.sync.dma_start(s_widx16[:], t["widx16"])

    # ---------------- token blocks ----------------
    inv_h = 1.0 / H
    for ch in range(BPC):
        wch = wpool.tile([128, 4, H], F32)
        if OPT_ACC_GATHER:
            # prefill with positional rows, then accumulate gathered words
            # (one [P,1]-indexed indirect gather per 128-token block)
            nc.sync.dma_start(wch[:], s_pos[:])
            for jj in range(4):
                nc.gpsimd.indirect_dma_start(
                    wch[:, jj, :], None, t["word_aug"],
                    bass.IndirectOffsetOnAxis(
                        ap=s_widx[:, 4 * ch + jj : 4 * ch + jj + 1], axis=0
                    ),
                    compute_op=ADD,
                )
        elif OPT_INDIRECT:
            nc.gpsimd.indirect_dma_start(
                wch[:], None, t["word_aug"],
                bass.IndirectOffsetOnAxis(
                    ap=s_widx[:, 4 * ch : 4 * (ch + 1)], axis=0
                ),
            )
        else:
            nc.gpsimd.dma_gather(
                wch[:], t["word_aug"], s_widx16[:, 32 * ch : 32 * (ch + 1)],
                512, 512, H,
            )
        for jj in range(4):
            j = ch * 4 + jj
            ps = psum.tile([128, H], F32)
            lhs_mh = s_mh[:, j * 128 : (j + 1) * 128]
            lhs_oh = s_oh[:, j * 128 : (j + 1) * 128]
            nc.tensor.matmul(
                ps[:, 0:512], lhs_mh, s_small[:, 0:512], start=True, stop=False
            )
            nc.tensor.matmul(
                ps[:, 0:512], lhs_oh, hadj_hi[:, 0:512], start=False, stop=False
            )
            nc.tensor.matmul(
                ps[:, 0:512], lhs_oh, hadj_lo[:, 0:512], start=False, stop=True
            )
            nc.tensor.matmul(
                ps[:, 512:H], lhs_mh, s_small[:, 512:H], start=True, stop=False
            )
            nc.tensor.matmul(
                ps[:, 512:H], lhs_oh, hadj_hi[:, 512:H], start=False, stop=False
            )
            nc.tensor.matmul(
                ps[:, 512:H], lhs_oh, hadj_lo[:, 512:H], start=False, stop=True
            )

            if OPT_ACC_GATHER:
                base = wch[:, jj, :]
            else:
                basetile = epool.tile([128, H], F32, tag="base")
                nc.vector.tensor_add(basetile[:], wch[:, jj, :], s_pos[:, jj, :])
                base = basetile[:]

            emb = epool.tile([128, H], F32)
            rsum = spool.tile([128, 1], F32)
            if OPT_TTR:
                # emb = (words+pos) + ps, and the row-sum, in one DVE pass
                nc.vector.tensor_tensor_reduce(
                    emb[:], base, ps[:], 1.0, 0.0, ADD, ADD, rsum[:]
                )
            else:
                nc.vector.tensor_add(emb[:], base, ps[:])

            u = spool.tile([128, 1], F32)
            var = spool.tile([128, 1], F32)
            if OPT_ACT_STATS:
                if not OPT_TTR:
                    # row sum on ACT (output write is a throwaway)
                    cp = qpool.tile([128, H], F32, tag="cp")
                    nc.scalar.activation(
                        cp[:], emb[:], AF.Copy, accum_out=rsum[:]
                    )
                # sum of squares on ACT
                sq = qpool.tile([128, H], F32)
                rsumsq = spool.tile([128, 1], F32)
                nc.scalar.activation(
                    sq[:], emb[:], AF.Square, accum_out=rsumsq[:]
                )
                # u, var  (tiny per-partition ops)
                nc.vector.tensor_scalar_mul(u[:], rsum[:], inv_h)
                mu2 = spool.tile([128, 1], F32)
                nc.vector.tensor_scalar(
                    mu2[:], u[:], u[:], -1.0, op0=MUL, op1=MUL
                )
                nc.vector.tensor_scalar(
                    var[:], rsumsq[:], inv_h, mu2[:], op0=MUL, op1=ADD
                )
            else:
                stats = spool.tile([128, 2, 6], F32)
                for g in range(2):
                    nc.vector.bn_stats(
                        stats[:, g, :], emb[:, g * 384 : (g + 1) * 384]
                    )
                mv = spool.tile([128, 2], F32)
                nc.vector.bn_aggr(mv[:], stats[:])
                nc.vector.tensor_copy(u[:], mv[:, 0:1])
                nc.vector.tensor_copy(var[:], mv[:, 1:2])

            rstd = spool.tile([128, 1], F32)
            nc.scalar.activation(
                rstd[:], var[:], AF.Sqrt, bias=s_eps[:], scale=1.0
            )
            nc.vector.reciprocal(rstd[:], rstd[:])

            o = opool.tile([128, H], F32)
            if OPT_ACT_APPLY:
                nub = spool.tile([128, 1], F32)
                nc.vector.tensor_scalar(
                    nub[:], u[:], rstd[:], -1.0, op0=MUL, op1=MUL
                )
                # LN apply on ACT: out = rstd*emb - u*rstd
                nc.scalar.activation(
                    o[:], emb[:], AF.Identity, bias=nub[:], scale=rstd[:]
                )
            else:
                nc.vector.tensor_scalar(
                    o[:], emb[:], u[:], rstd[:],
                    op0=mybir.AluOpType.subtract, op1=MUL,
                )
            if not skip_affine:
                nc.vector.tensor_mul(o[:], o[:], s_lnw[:])
                nc.vector.tensor_add(o[:], o[:], s_lnb[:])

            nc.sync.dma_start(out[ch, jj * 128 : (jj + 1) * 128, :], o[:])


def _wrap16(flat):
    w = flat.reshape(-1, 16).T.astype(np.int16)
    return np.tile(w, (8, 1))


def _prep_core(core, iid, hdr, tt, mt, ti, cpos, cidx, hlen):
    import ml_dtypes

    b0 = core * BPC
    sl = slice(b0, b0 + BPC)
    iids = iid[sl]

    # word gather rows in (p, ch*4+j) layout for the indirect gathers
    widx = np.ascontiguousarray(
        iids.reshape(BPC * 4, 128).T.astype(np.int32)
    )  # [128, NBLK] ; widx[p, j] = token j*128+p

    bb = np.arange(BPC)[:, None]
    sel_hdr = hdr[sl][bb, cidx[sl]]                      # [BPC, C, L]
    sel_len = hlen[sl][bb, cidx[sl]]                     # [BPC, C]
    maskl = np.arange(L)[None, None, :] < sel_len[:, :, None]
    hvals = np.where(maskl, sel_hdr, ZROW)               # [BPC, C, L]
    hflat = hvals.reshape(NSLOT, L).T.reshape(-1)        # i2 = l*128 + slot
    hidx = _wrap16(hflat)

    # word rows at each slot's target position
    wtidx = iids[bb, cpos[sl]].reshape(NSLOT, 1).astype(np.int32)

    # one-hot scatter matrix [128 slots, T] (bf16): column = local target token
    tgt = np.where(
        sel_len.reshape(-1) > 0, (bb * S + cpos[sl]).reshape(-1), -1
    )
    oh = np.zeros((NSLOT, T), dtype=ml_dtypes.bfloat16)
    valid = tgt >= 0
    oh[np.arange(NSLOT)[valid], tgt[valid]] = 1

    # multi-hot small-table encoding [19, T], stacked twice for the
    # hi/lo error-compensated small-table matmul
    mh1 = np.zeros((NV, T), dtype=ml_dtypes.bfloat16)
    ar = np.arange(T)
    mh1[tt[sl].reshape(-1), ar] = 1
    mh1[2 + mt[sl].reshape(-1), ar] += 1
    mh1[13 + ti[sl].reshape(-1), ar] += 1
    mh = np.concatenate([mh1, mh1], axis=0)

    hl = sel_len.reshape(NSLOT, 1).astype(np.float32)
    widx16 = _wrap16(iids.reshape(-1))
    wtidx16 = _wrap16(wtidx.reshape(-1))
    return widx, wtidx, hidx, oh, mh, hl, widx16, wtidx16


def make_in_maps(inputs):
    import ml_dtypes

    inp = {k: np.asarray(v) for k, v in inputs.items()}
    word = np.ascontiguousarray(inp["word_emb"], dtype=np.float32)
    word_aug = np.concatenate([word, np.zeros((1, H), np.float32)], axis=0)
    small3_f32 = np.concatenate(
        [inp["tok_type_emb"], inp["match_emb"], inp["type_emb"]], axis=0
    ).astype(np.float32)
    small_hi = small3_f32.astype(ml_dtypes.bfloat16)
    small_lo = (small3_f32 - small_hi.astype(np.float32)).astype(
        ml_dtypes.bfloat16
    )
    small3 = np.concatenate([small_hi, small_lo], axis=0)
    pos = np.ascontiguousarray(inp["pos_emb"], dtype=np.float32)
    lnw = np.ascontiguousarray(inp["ln_w"], dtype=np.float32).reshape(1, H)
    lnb = np.ascontiguousarray(inp["ln_b"], dtype=np.float32).reshape(1, H)
    skip_affine = bool(np.all(lnw == 1.0) and np.all(lnb == 0.0))

    iid = inp["input_ids"].astype(np.int64)
    hdr = inp["header_ids"].astype(np.int64)
    tt = inp["token_type_ids"].astype(np.int64)
    mt = inp["match_type_ids"].astype(np.int64)
    ti = inp["type_idx"].astype(np.int64)
    cpos = inp["col_pos"].astype(np.int64)
    cidx = inp["col_idx"].astype(np.int64)
    hlen = inp["header_len"].astype(np.int64)

    in_maps = []
    for core in range(NCORES):
        widx, wtidx, hidx, oh, mh, hl, widx16, wtidx16 = _prep_core(
            core, iid, hdr, tt, mt, ti, cpos, cidx, hlen
        )
        m = dict(
            word_aug=word_aug, pos_emb=pos, small3=small3,
            mh=mh, oh=oh, hl=hl, widx=widx, wtidx=wtidx, hidx=hidx,
            widx16=widx16, wtidx16=wtidx16,
        )
        if not skip_affine:
            m["lnw"] = lnw
            m["lnb"] = lnb
        in_maps.append(m)
    return in_maps, skip_affine


def get_nc(skip_affine):
    if skip_affine not in _NC_CACHE:
        _NC_CACHE[skip_affine] = _build_nc(skip_affine)
    return _NC_CACHE[skip_affine]


def run_hw(inputs, trace=False, trace_cores=None):
    """Returns (out [B,S,H] f32, BassKernelResults)."""
    from concourse.bass_utils import run_bass_kernel_spmd

    in_maps, skip_affine = make_in_maps(inputs)
    nc = get_nc(skip_affine)
    res = run_bass_kernel_spmd(
        nc, in_maps, core_ids=list(range(NCORES)), trace=trace,
        trace_cores=trace_cores,
    )
    out = np.concatenate([res.results[c]["out"] for c in range(NCORES)], axis=0)
    return out, res


def kernel(**inputs) -> np.ndarray:
    out, _ = run_hw(inputs, trace=False)
    return out
